# revision 9
# baseline (speedup 1.0000x reference)
"""Causal single-head attention (B=4, T=2048, D=1024, fp32) on 8 TRN2 NeuronCores.

Sharding: 2 cores per batch. Within a pair, keys/values are split by
interleaved 128-token tiles (core parity p takes s-tiles t with t%2==p), which
makes the program perfectly uniform across cores (one SPMD program, per-core
differences live entirely in the input data): for every 512-wide query chunk
i, each core processes exactly 2i+2 local key tiles, with the causal boundary
applied through two per-core additive mask tiles. Each core computes an
unnormalized partial attention output plus softmax denominators for ALL
queries of its batch; the host merges the two partials per batch (add, then
divide) while unsharding.

v2 (this file) vs the fp32r baseline (195.7us):
- All matmul operands are bf16 (fp32 PSUM accumulation): halves every DMA
  byte, enables FWL weight loads (LDWEIGHTS 53ns vs 215ns fp32r, fully
  hidden by the PE reorder window), and halves DVE copy write bytes.
  Logit abs err ~0.01 -> end-to-end rel err ~1e-3, comfortably under the
  2e-2 gate.
- All inputs are loaded up-front into pinned SBUF tiles (everything fits in
  bf16: ~122KB/partition of 208KB) with one 256-512KB DMA per k-tile, issued
  in consumption order and alternating between the two HWDGE rings (SP/ACT).
  The fp32r baseline staged tiles lazily per phase: its ~187 dma triggers at
  ~0.61us sequencer cost each backlogged the rings, and the PE stalled
  ~20us at phase boundaries (plus ~20us of HAM half-clock penalties from
  the >3.4us idle windows those stalls opened).
- Warmup matmuls run on a DVE-memset tile (the gpsimd memset of the baseline
  took ~5us before the first warm matmul could issue).
- Output partials are written in bf16 (262KB per query block), one trigger
  per block mid-kernel, split 2-4 ways only for the final chunk's blocks
  where the drain is the kernel tail.

Softmax runs without max-subtraction: logits = scores/32 stay within ~+-8
for this input distribution, far from overflow in fp32 PSUM / bf16 exp.
"""
import numpy as np

B, T, D = 4, 2048, 1024
P = 128
NK = D // P          # 8 contraction tiles
QC = T // 512        # 4 query chunks of 512
NEG = -1e30
SCALE = 1.0 / 32.0   # 1/sqrt(D)
N_WARM = 20

_prog = None
_last_in_maps = None


def _build_program():
    import concourse.bacc as bacc
    import concourse.mybir as mybir
    import concourse.tile as tile

    f32 = mybir.dt.float32
    bf = mybir.dt.bfloat16

    nc = bacc.Bacc()
    xt_d = nc.declare_dram_parameter("xt", [D, T], bf, isOutput=False)
    xtl_d = nc.declare_dram_parameter("xtl", [D, T // 2], bf, isOutput=False)
    wkq_d = nc.declare_dram_parameter("wkq", [D, D], bf, isOutput=False)
    wv_d = nc.declare_dram_parameter("wv", [D, D], bf, isOutput=False)
    mask_d = nc.declare_dram_parameter("masks", [2, P, 512], bf, isOutput=False)
    ones_d = nc.declare_dram_parameter("ones", [P, 2], bf, isOutput=False)
    part_d = nc.declare_dram_parameter("part", [T, D + 1], bf, isOutput=True)

    with tile.TileContext(nc) as tc:
        with tc.tile_pool(name="sbuf", bufs=1) as pool, \
             tc.tile_pool(name="psum", bufs=1, space="PSUM") as psum:

            # Alternate dma issue between the two HWDGE rings (SP/ACT):
            # each trigger costs ~0.61us on its issuing sequencer.
            _eng = [0]

            def dma(dst, src_ap):
                e = nc.sync if _eng[0] % 2 == 0 else nc.scalar
                _eng[0] += 1
                e.dma_start(dst, src_ap)

            # ---- pinned input/working tiles (all bf16) ----
            wq_sb = pool.tile([P, NK, D], bf, tag="wq")        # Wk@Wq^T rows
            xl_sb = pool.tile([P, NK, T // 2], bf, tag="xl")   # local s cols of x^T
            wv_sb = pool.tile([P, NK, D], bf, tag="wv")
            xt_sb = pool.tile([P, NK, T], bf, tag="xt")        # all queries, natural
            kt_sb = pool.tile([P, NK, T // 2], bf, tag="kt")   # K'^T, local s
            v_sb = pool.tile([P, NK, D], bf, tag="v")          # V, local s tiles
            mask_t = pool.tile([P, 2, 512], bf, tag="mask")
            ones_t = pool.tile([P, 2], bf, tag="ones")

            # ---- HAM pre-warm ----
            # PE sits behind the DMA load window at kernel start; throwaway
            # matmuls on a DVE-memset tile hold the clock gate at 8/8.
            # The warmup must cover until the critical-load k-chains are fed
            # (~20us): the first k-pair chains trickle in between warm MMs
            # ending and full feed, and any >3.4us idle window re-throttles
            # the PE clock to 1.2GHz for ~7us.
            warm = pool.tile([P, 512], bf, tag="warm")
            nc.vector.memset(warm[:], 0.0)
            wps = psum.tile([P, 512], f32, tag="ps512", bufs=2)
            for w in range(N_WARM):
                nc.tensor.matmul(wps[:], warm[:, 0:P], warm[:],
                                 start=(w == 0), stop=(w == N_WARM - 1))

            # ---- up-front loads, in consumption order ----
            # Multi-k-tile triggers: one dma_start covers [P, ks, cols] via a
            # rearranged DRAM AP, so all 16 SDMA engines fan out immediately
            # (per-k 128KB triggers fed the first chain only by ~17us; these
            # land it ~15us with 15 fewer issue slots). Phase B's first chain
            # (h=0, j=0) needs the h0 half-rows of wkq + j0 half-rows of xtl:
            # that critical 2MB goes first, split k0-3/k4-7 for pipelining,
            # wkq on SP and xtl on ACT.
            def big(dst_tile, src, k0, k1, c0, c1):
                dma(dst_tile[:, k0:k1, c0:c1],
                    src[k0 * P:k1 * P, c0:c1].rearrange("(k p) c -> p k c", p=P))

            big(wq_sb, wkq_d, 0, 4, 0, 512)
            big(xl_sb, xtl_d, 0, 4, 0, 512)
            big(wq_sb, wkq_d, 4, 8, 0, 512)
            big(xl_sb, xtl_d, 4, 8, 0, 512)
            big(xl_sb, xtl_d, 0, 8, 512, 1024)   # phase B j=1
            big(wq_sb, wkq_d, 0, 8, 512, 1024)   # phase B h=1
            big(wv_sb, wv_d, 0, 8, 0, 512)       # phase C n=0
            big(wv_sb, wv_d, 0, 8, 512, 1024)    # phase C n=1
            dma(mask_t[:, 0, :], mask_d[0])
            dma(mask_t[:, 1, :], mask_d[1])
            dma(ones_t[:], ones_d[:])
            for i in range(QC):                  # phase D chunk i
                big(xt_sb, xt_d, 0, 8, 512 * i, 512 * (i + 1))

            # ---- phase B: K'^T = (Wk Wq^T)^T x^T over local s ----
            for h in range(2):                     # dout halves
                for j in range(2):                 # local s 512-chunks
                    for mm in range(4):
                        m = 4 * h + mm
                        c = 512 * h + mm * P
                        ps = psum.tile([P, 512], f32, tag="ps512", bufs=2)
                        for k in range(NK):
                            nc.tensor.matmul(ps[:], wq_sb[:, k, c:c + P],
                                             xl_sb[:, k, 512 * j:512 * (j + 1)],
                                             start=(k == 0), stop=(k == NK - 1))
                        nc.vector.tensor_copy(kt_sb[:, m, 512 * j:512 * (j + 1)], ps[:])

            # ---- phase C: V over local s ----
            for n in range(2):                     # dv halves
                for j in range(2):
                    for lt4 in range(4):           # local 128-tiles in chunk j
                        lt = 4 * j + lt4
                        ps = psum.tile([P, 512], f32, tag="ps512", bufs=2)
                        for k in range(NK):
                            nc.tensor.matmul(ps[:],
                                             xl_sb[:, k, (4 * j + lt4) * P:(4 * j + lt4 + 1) * P],
                                             wv_sb[:, k, 512 * n:512 * (n + 1)],
                                             start=(k == 0), stop=(k == NK - 1))
                        nc.vector.tensor_copy(v_sb[:, lt, 512 * n:512 * (n + 1)], ps[:])

            # ---- phase D: per query chunk ----
            # scores fold the Q projection into the host-precomputed wkq, so
            # the S^T matmul consumes resident x^T columns.
            for i in range(QC):
                nlt_all = 2 * i + 2
                pt = pool.tile([P, NK, 512], bf, tag="pt", bufs=2)
                for lt in range(nlt_all):
                    # the last local tile (lt == 2i+1) is fully masked for the
                    # first 256 query columns AND excluded from their attn@V
                    # accumulation (nlt), so only its right half is computed
                    lo = 256 if lt == 2 * i + 1 else 0
                    ps = psum.tile([P, 512 - lo], f32, tag="ps512", bufs=2)
                    for m in range(NK):
                        nc.tensor.matmul(ps[:], kt_sb[:, m, lt * P:(lt + 1) * P],
                                         xt_sb[:, m, 512 * i + lo:512 * (i + 1)],
                                         start=(m == 0), stop=(m == NK - 1))
                    if lt == 2 * i:
                        nc.vector.tensor_add(ps[:], ps[:], mask_t[:, 0, :])
                    elif lt == 2 * i + 1:
                        nc.vector.tensor_add(ps[:], ps[:], mask_t[:, 1, 256:512])
                    nc.scalar.activation(pt[:, lt, lo:512], ps[:],
                                         mybir.ActivationFunctionType.Exp,
                                         bias=0.0, scale=SCALE)

                qb_order = [3, 2, 1, 0] if i == QC - 1 else [0, 1, 2, 3]
                for qb in qb_order:
                    nlt = 2 * i + 1 if qb < 2 else 2 * i + 2
                    pso = psum.tile([P, D], f32, tag="psO", bufs=2)
                    pss = psum.tile([P, 2], f32, tag="psS", bufs=2)
                    for t_ in range(nlt):
                        lhs = pt[:, t_, qb * P:(qb + 1) * P]
                        st, sp = (t_ == 0), (t_ == nlt - 1)
                        nc.tensor.matmul(pso[:, 0:512], lhs, v_sb[:, t_, 0:512],
                                         start=st, stop=sp)
                        nc.tensor.matmul(pso[:, 512:1024], lhs, v_sb[:, t_, 512:1024],
                                         start=st, stop=sp)
                        nc.tensor.matmul(pss[:], lhs, ones_t[:], start=st, stop=sp)
                    osb = pool.tile([P, D + 1], bf, tag="osb", bufs=2)
                    r0 = 512 * i + qb * P
                    if i < QC - 1:
                        # 262KB bf16 per block: one trigger mid-kernel
                        nc.vector.tensor_copy(osb[:, 0:D], pso[:])
                        nc.vector.tensor_copy(osb[:, D:D + 1], pss[:, 0:1])
                        dma(part_d[r0:r0 + P, :], osb[:])
                    else:
                        # tail: pipeline copy quarters with their DMAs so the
                        # drain starts ~400ns after the last matmul
                        nc.vector.tensor_copy(osb[:, D:D + 1], pss[:, 0:1])
                        for c4 in range(4):
                            c_lo = c4 * 256
                            c_hi = D + 1 if c4 == 3 else c_lo + 256
                            nc.vector.tensor_copy(osb[:, c_lo:c_lo + 256],
                                                  pso[:, c_lo:c_lo + 256])
                            dma(part_d[r0:r0 + P, c_lo:c_hi], osb[:, c_lo:c_hi])

    nc.finalize()
    return nc


def _get_program():
    global _prog
    if _prog is None:
        _prog = _build_program()
    return _prog


def kernel(x, Wq, Wk, Wv):
    import ml_dtypes
    from concourse.bass_utils import run_bass_kernel_spmd

    bf16 = ml_dtypes.bfloat16
    x = np.asarray(x, dtype=np.float32)
    Wq = np.ascontiguousarray(np.asarray(Wq, dtype=np.float32))
    Wk = np.ascontiguousarray(np.asarray(Wk, dtype=np.float32))
    Wv = np.ascontiguousarray(np.asarray(Wv, dtype=np.float32))

    ones = np.ones((P, 2), dtype=bf16)
    # scores = x (Wq Wk^T) x^T: fold the two projection matrices on the host.
    # The device tensor plays the old Wk role: lhsT[b, a] = (Wk Wq^T)[b, a].
    Wkq = np.ascontiguousarray(
        (Wk.astype(np.float64) @ Wq.T.astype(np.float64)).astype(np.float32)
    ).astype(bf16)
    Wv_b = np.ascontiguousarray(Wv).astype(bf16)
    sr = np.arange(P)[:, None]
    qr = np.arange(512)[None, :]
    masks = {}
    for p in (0, 1):
        m0 = np.where(128 * p + sr > qr, NEG, 0.0).astype(bf16)
        m1 = np.where(128 * (2 + p) + sr > qr, NEG, 0.0).astype(bf16)
        masks[p] = np.stack([m0, m1])

    in_maps = []
    for c in range(8):
        b, p = c // 2, c % 2
        xt = np.ascontiguousarray(x[b].T)                     # [D, T]
        xtv = xt.reshape(D, T // P, P)
        xtl = np.ascontiguousarray(
            xtv[:, p::2, :].reshape(D, T // 2)).astype(bf16)  # local s cols
        in_maps.append({
            "xt": xt.astype(bf16), "xtl": xtl,
            "wkq": Wkq, "wv": Wv_b,
            "masks": masks[p], "ones": ones,
        })

    global _last_in_maps
    _last_in_maps = in_maps
    nc = _get_program()
    res = run_bass_kernel_spmd(nc, in_maps, list(range(8)))

    out = np.empty((B, T, D), dtype=np.float32)
    for b in range(B):
        p0 = res.results[2 * b]["part"].astype(np.float32)
        p1 = res.results[2 * b + 1]["part"].astype(np.float32)
        O = p0[:, :D] + p1[:, :D]
        d = p0[:, D] + p1[:, D]
        out[b] = O / d[:, None]
    return out


# revision 12
# speedup vs baseline: 1.0166x; 1.0166x over previous
"""Causal single-head attention (B=4, T=2048, D=1024, fp32) on 8 TRN2 NeuronCores.

Sharding: 2 cores per batch. Within a pair, keys/values are split by
interleaved 128-token tiles (core parity p takes s-tiles t with t%2==p), which
makes the program perfectly uniform across cores (one SPMD program, per-core
differences live entirely in the input data): for every 512-wide query chunk
i, each core processes exactly 2i+2 local key tiles, with the causal boundary
applied through two per-core additive mask tiles. Each core computes an
unnormalized partial attention output plus softmax denominators for ALL
queries of its batch; the host merges the two partials per batch (add, then
divide) while unsharding.

v2 (this file) vs the fp32r baseline (195.7us):
- All matmul operands are bf16 (fp32 PSUM accumulation): halves every DMA
  byte, enables FWL weight loads (LDWEIGHTS 53ns vs 215ns fp32r, fully
  hidden by the PE reorder window), and halves DVE copy write bytes.
  Logit abs err ~0.01 -> end-to-end rel err ~1e-3, comfortably under the
  2e-2 gate.
- All inputs are loaded up-front into pinned SBUF tiles (everything fits in
  bf16: ~122KB/partition of 208KB) with one 256-512KB DMA per k-tile, issued
  in consumption order and alternating between the two HWDGE rings (SP/ACT).
  The fp32r baseline staged tiles lazily per phase: its ~187 dma triggers at
  ~0.61us sequencer cost each backlogged the rings, and the PE stalled
  ~20us at phase boundaries (plus ~20us of HAM half-clock penalties from
  the >3.4us idle windows those stalls opened).
- Warmup matmuls run on a DVE-memset tile (the gpsimd memset of the baseline
  took ~5us before the first warm matmul could issue).
- Output partials are written in bf16 (262KB per query block), one trigger
  per block mid-kernel, split 2-4 ways only for the final chunk's blocks
  where the drain is the kernel tail.

Softmax runs without max-subtraction: logits = scores/32 stay within ~+-8
for this input distribution, far from overflow in fp32 PSUM / bf16 exp.
"""
import numpy as np

B, T, D = 4, 2048, 1024
P = 128
NK = D // P          # 8 contraction tiles
QC = T // 512        # 4 query chunks of 512
NEG = -1e30
SCALE = 1.0 / 32.0   # 1/sqrt(D)
N_WARM = 24

_prog = None
_last_in_maps = None


def _build_program():
    import concourse.bacc as bacc
    import concourse.mybir as mybir
    import concourse.tile as tile

    f32 = mybir.dt.float32
    bf = mybir.dt.bfloat16

    nc = bacc.Bacc()
    xt_d = nc.declare_dram_parameter("xt", [D, T], bf, isOutput=False)
    xtl_d = nc.declare_dram_parameter("xtl", [D, T // 2], bf, isOutput=False)
    wkq_d = nc.declare_dram_parameter("wkq", [D, D], bf, isOutput=False)
    wv_d = nc.declare_dram_parameter("wv", [D, D], bf, isOutput=False)
    mask_d = nc.declare_dram_parameter("masks", [2, P, 512], bf, isOutput=False)
    ones_d = nc.declare_dram_parameter("ones", [P, 2], bf, isOutput=False)
    part_d = nc.declare_dram_parameter("part", [T, D + 1], bf, isOutput=True)

    with tile.TileContext(nc) as tc:
        with tc.tile_pool(name="sbuf", bufs=1) as pool, \
             tc.tile_pool(name="psum", bufs=1, space="PSUM") as psum:

            # Alternate dma issue between the two HWDGE rings (SP/ACT):
            # each trigger costs ~0.61us on its issuing sequencer.
            _eng = [0]

            def dma(dst, src_ap):
                e = nc.sync if _eng[0] % 2 == 0 else nc.scalar
                _eng[0] += 1
                e.dma_start(dst, src_ap)

            # ---- pinned input/working tiles (all bf16) ----
            wq_sb = pool.tile([P, NK, D], bf, tag="wq")        # Wk@Wq^T rows
            xl_sb = pool.tile([P, NK, T // 2], bf, tag="xl")   # local s cols of x^T
            wv_sb = pool.tile([P, NK, D], bf, tag="wv")
            xt_sb = pool.tile([P, NK, T], bf, tag="xt")        # all queries, natural
            kt_sb = pool.tile([P, NK, T // 2], bf, tag="kt")   # K'^T, local s
            v_sb = pool.tile([P, NK, D], bf, tag="v")          # V, local s tiles
            mask_t = pool.tile([P, 2, 512], bf, tag="mask")
            ones_t = pool.tile([P, 2], bf, tag="ones")

            # ---- HAM pre-warm ----
            # PE sits behind the DMA load window at kernel start; throwaway
            # matmuls on a DVE-memset tile hold the clock gate at 8/8.
            # The warmup must cover until the critical-load k-chains are fed
            # (~20us): the first k-pair chains trickle in between warm MMs
            # ending and full feed, and any >3.4us idle window re-throttles
            # the PE clock to 1.2GHz for ~7us.
            warm = pool.tile([P, 512], bf, tag="warm")
            nc.vector.memset(warm[:], 0.0)
            wps = psum.tile([P, 512], f32, tag="ps512", bufs=2)
            for w in range(N_WARM):
                nc.tensor.matmul(wps[:], warm[:, 0:P], warm[:],
                                 start=(w == 0), stop=(w == N_WARM - 1))

            # ---- up-front loads, in consumption order ----
            # Multi-k-tile triggers: one dma_start covers [P, ks, cols] via a
            # rearranged DRAM AP, so all 16 SDMA engines fan out immediately
            # (per-k 128KB triggers fed the first chain only by ~17us; these
            # land it ~15us with 15 fewer issue slots). Phase B's first chain
            # (h=0, j=0) needs the h0 half-rows of wkq + j0 half-rows of xtl:
            # that critical 2MB goes first, split k0-3/k4-7 for pipelining,
            # wkq on SP and xtl on ACT.
            def big(dst_tile, src, k0, k1, c0, c1):
                dma(dst_tile[:, k0:k1, c0:c1],
                    src[k0 * P:k1 * P, c0:c1].rearrange("(k p) c -> p k c", p=P))

            big(wq_sb, wkq_d, 0, 4, 0, 512)
            big(xl_sb, xtl_d, 0, 4, 0, 512)
            big(wq_sb, wkq_d, 4, 6, 0, 512)
            big(xl_sb, xtl_d, 4, 6, 0, 512)
            big(wq_sb, wkq_d, 6, 8, 0, 512)
            big(xl_sb, xtl_d, 6, 8, 0, 512)
            big(xl_sb, xtl_d, 0, 8, 512, 1024)   # phase B j=1
            big(wq_sb, wkq_d, 0, 8, 512, 1024)   # phase B h=1
            big(wv_sb, wv_d, 0, 8, 0, 512)       # phase C n=0
            big(wv_sb, wv_d, 0, 8, 512, 1024)    # phase C n=1
            dma(mask_t[:, 0, :], mask_d[0])
            dma(mask_t[:, 1, :], mask_d[1])
            dma(ones_t[:], ones_d[:])
            for i in range(QC):                  # phase D chunk i
                big(xt_sb, xt_d, 0, 8, 512 * i, 512 * (i + 1))

            # ---- phase B: K'^T = (Wk Wq^T)^T x^T over local s ----
            for h in range(2):                     # dout halves
                for j in range(2):                 # local s 512-chunks
                    for mm in range(4):
                        m = 4 * h + mm
                        c = 512 * h + mm * P
                        ps = psum.tile([P, 512], f32, tag="ps512", bufs=2)
                        for k in range(NK):
                            nc.tensor.matmul(ps[:], wq_sb[:, k, c:c + P],
                                             xl_sb[:, k, 512 * j:512 * (j + 1)],
                                             start=(k == 0), stop=(k == NK - 1))
                        nc.vector.tensor_copy(kt_sb[:, m, 512 * j:512 * (j + 1)], ps[:])

            # ---- phase C: V over local s ----
            for n in range(2):                     # dv halves
                for j in range(2):
                    for lt4 in range(4):           # local 128-tiles in chunk j
                        lt = 4 * j + lt4
                        ps = psum.tile([P, 512], f32, tag="ps512", bufs=2)
                        for k in range(NK):
                            nc.tensor.matmul(ps[:],
                                             xl_sb[:, k, (4 * j + lt4) * P:(4 * j + lt4 + 1) * P],
                                             wv_sb[:, k, 512 * n:512 * (n + 1)],
                                             start=(k == 0), stop=(k == NK - 1))
                        nc.vector.tensor_copy(v_sb[:, lt, 512 * n:512 * (n + 1)], ps[:])

            # ---- phase D: per query chunk ----
            # scores fold the Q projection into the host-precomputed wkq, so
            # the S^T matmul consumes resident x^T columns.
            for i in range(QC):
                nlt_all = 2 * i + 2
                pt = pool.tile([P, NK, 512], bf, tag="pt", bufs=2)
                for lt in range(nlt_all):
                    # the last local tile (lt == 2i+1) is fully masked for the
                    # first 256 query columns AND excluded from their attn@V
                    # accumulation (nlt), so only its right half is computed
                    lo = 256 if lt == 2 * i + 1 else 0
                    ps = psum.tile([P, 512 - lo], f32, tag="ps512", bufs=2)
                    for m in range(NK):
                        nc.tensor.matmul(ps[:], kt_sb[:, m, lt * P:(lt + 1) * P],
                                         xt_sb[:, m, 512 * i + lo:512 * (i + 1)],
                                         start=(m == 0), stop=(m == NK - 1))
                    if lt == 2 * i:
                        nc.vector.tensor_add(ps[:], ps[:], mask_t[:, 0, :])
                    elif lt == 2 * i + 1:
                        nc.vector.tensor_add(ps[:], ps[:], mask_t[:, 1, 256:512])
                    nc.scalar.activation(pt[:, lt, lo:512], ps[:],
                                         mybir.ActivationFunctionType.Exp,
                                         bias=0.0, scale=SCALE)

                qb_order = [3, 2, 1, 0] if i == QC - 1 else [0, 1, 2, 3]
                for qb in qb_order:
                    nlt = 2 * i + 1 if qb < 2 else 2 * i + 2
                    pso = psum.tile([P, D], f32, tag="psO", bufs=2)
                    pss = psum.tile([P, 2], f32, tag="psS", bufs=2)
                    for t_ in range(nlt):
                        lhs = pt[:, t_, qb * P:(qb + 1) * P]
                        st, sp = (t_ == 0), (t_ == nlt - 1)
                        nc.tensor.matmul(pso[:, 0:512], lhs, v_sb[:, t_, 0:512],
                                         start=st, stop=sp)
                        nc.tensor.matmul(pso[:, 512:1024], lhs, v_sb[:, t_, 512:1024],
                                         start=st, stop=sp)
                        nc.tensor.matmul(pss[:], lhs, ones_t[:], start=st, stop=sp)
                    osb = pool.tile([P, D + 1], bf, tag="osb", bufs=2)
                    r0 = 512 * i + qb * P
                    # output triggers go on SP only: on the ACT ring they
                    # queue ahead of the next chunk's Exp activations (FIFO)
                    # and stall PSUM recycling for ~1us per chunk boundary
                    if i < QC - 1:
                        # 262KB bf16 per block: one trigger mid-kernel
                        nc.vector.tensor_copy(osb[:, 0:D], pso[:])
                        nc.vector.tensor_copy(osb[:, D:D + 1], pss[:, 0:1])
                        nc.sync.dma_start(part_d[r0:r0 + P, :], osb[:])
                    else:
                        # tail: pipeline copy quarters with their DMAs so the
                        # drain starts ~400ns after the last matmul
                        nc.vector.tensor_copy(osb[:, D:D + 1], pss[:, 0:1])
                        for c4 in range(4):
                            c_lo = c4 * 256
                            c_hi = D + 1 if c4 == 3 else c_lo + 256
                            nc.vector.tensor_copy(osb[:, c_lo:c_lo + 256],
                                                  pso[:, c_lo:c_lo + 256])
                            nc.sync.dma_start(part_d[r0:r0 + P, c_lo:c_hi],
                                              osb[:, c_lo:c_hi])

    nc.finalize()
    return nc


def _get_program():
    global _prog
    if _prog is None:
        _prog = _build_program()
    return _prog


def kernel(x, Wq, Wk, Wv):
    import ml_dtypes
    from concourse.bass_utils import run_bass_kernel_spmd

    bf16 = ml_dtypes.bfloat16
    x = np.asarray(x, dtype=np.float32)
    Wq = np.ascontiguousarray(np.asarray(Wq, dtype=np.float32))
    Wk = np.ascontiguousarray(np.asarray(Wk, dtype=np.float32))
    Wv = np.ascontiguousarray(np.asarray(Wv, dtype=np.float32))

    ones = np.ones((P, 2), dtype=bf16)
    # scores = x (Wq Wk^T) x^T: fold the two projection matrices on the host.
    # The device tensor plays the old Wk role: lhsT[b, a] = (Wk Wq^T)[b, a].
    Wkq = np.ascontiguousarray(
        (Wk.astype(np.float64) @ Wq.T.astype(np.float64)).astype(np.float32)
    ).astype(bf16)
    Wv_b = np.ascontiguousarray(Wv).astype(bf16)
    sr = np.arange(P)[:, None]
    qr = np.arange(512)[None, :]
    masks = {}
    for p in (0, 1):
        m0 = np.where(128 * p + sr > qr, NEG, 0.0).astype(bf16)
        m1 = np.where(128 * (2 + p) + sr > qr, NEG, 0.0).astype(bf16)
        masks[p] = np.stack([m0, m1])

    in_maps = []
    for c in range(8):
        b, p = c // 2, c % 2
        xt = np.ascontiguousarray(x[b].T)                     # [D, T]
        xtv = xt.reshape(D, T // P, P)
        xtl = np.ascontiguousarray(
            xtv[:, p::2, :].reshape(D, T // 2)).astype(bf16)  # local s cols
        in_maps.append({
            "xt": xt.astype(bf16), "xtl": xtl,
            "wkq": Wkq, "wv": Wv_b,
            "masks": masks[p], "ones": ones,
        })

    global _last_in_maps
    _last_in_maps = in_maps
    nc = _get_program()
    res = run_bass_kernel_spmd(nc, in_maps, list(range(8)))

    out = np.empty((B, T, D), dtype=np.float32)
    for b in range(B):
        p0 = res.results[2 * b]["part"].astype(np.float32)
        p1 = res.results[2 * b + 1]["part"].astype(np.float32)
        O = p0[:, :D] + p1[:, :D]
        d = p0[:, D] + p1[:, D]
        out[b] = O / d[:, None]
    return out
